# revision 1
# baseline (speedup 1.0000x reference)
"""Trainium2 Bass kernel for nn_AutoregressiveAllocPolicy (B=4096, NA=NT=16, D=128).

Math per batch elem b, agent step s:
  logits_k = dot(ag_s, te_k + nonag_k*W0 + counts_k*W1 + b_cnt) / sqrt(D)
  k* = argmax(logits + gumbel_s); out[s] = one_hot(k*)
  counts[k*] += 0.1;  te[k*] += relu([te[k*]; ag_s]) @ W_upd + b_upd

Exploited structure:
  - forward output is exactly one_hot(argmax)  (hard - sg(soft) + soft)
  - b_cnt shifts every k equally -> drop (argmax invariant)
  - te update touches one row/step -> te rows live in DRAM; selected rows
    move via dma_gather / dma_scatter_add (data-dependent row indices)
  - score state SCB[b,t,k] = dot(ag_t, te_cur[b,k])/sqrt(D) kept incrementally:
    initialized host-side (tiny einsum), then per-step corrections add
    dot(ag_t', upd) deltas via one-hot mask multiplies (no engine gathers).

Layout per core: 512 batch elems, b_local = g*128 + p (p partition, g=0..3).
"""
import sys
sys.path.insert(0, '/opt/trn_rl_repo')
import contextlib
import numpy as np

from concourse import bass, mybir, bacc, tile, bass_utils
from concourse.ap import AP

B, NA, NT, D = 4096, 16, 16, 128
CORES = 8
BS = B // CORES          # 512
G = BS // 128            # 4
INV_SCALE = float(1.0 / np.sqrt(np.float32(D)))
CNF = 0.1
F32 = mybir.dt.float32
I16 = mybir.dt.int16
ALU = None  # set after import in _build

_CACHE = {}


def _build(n_steps=NA, skip_corr=False, skip_lazy=False):
    alu = mybir.AluOpType
    act = mybir.ActivationFunctionType
    nc = bacc.Bacc("TRN2", target_bir_lowering=False, debug=False,
                   num_devices=CORES)

    d_terows = nc.dram_tensor("terows", [BS * NT, D], F32, kind="ExternalInput")
    d_dot0 = nc.dram_tensor("dot0", [128, G * NA * NT], F32, kind="ExternalInput")
    d_a01 = nc.dram_tensor("a01", [128, 2 * G * NA], F32, kind="ExternalInput")
    d_agt = nc.dram_tensor("agt", [128, G * 128 * NA], F32, kind="ExternalInput")
    d_agb = nc.dram_tensor("agb", [128, G * NA * D], F32, kind="ExternalInput")
    d_gg = nc.dram_tensor("gg", [128, G * NA * NT], F32, kind="ExternalInput")
    d_nonag = nc.dram_tensor("nonag", [128, G * NT], F32, kind="ExternalInput")
    d_wct = nc.dram_tensor("wct", [128, 2], F32, kind="ExternalInput")
    d_w1 = nc.dram_tensor("w1", [128, 128], F32, kind="ExternalInput")
    d_w2 = nc.dram_tensor("w2", [128, 128], F32, kind="ExternalInput")
    d_bupd = nc.dram_tensor("bupd", [128, 1], F32, kind="ExternalInput")
    d_iotak = nc.dram_tensor("iotak", [128, NT], F32, kind="ExternalInput")
    d_bc16 = nc.dram_tensor("bc16", [128, G], F32, kind="ExternalInput")
    d_ident = nc.dram_tensor("ident", [128, 128], F32, kind="ExternalInput")
    d_out = nc.dram_tensor("out", [128, G * NA * NT], F32, kind="ExternalOutput")
    d_tework = nc.dram_tensor("tework", [BS * NT, D], F32)

    with tile.TileContext(nc) as tc:
        with contextlib.ExitStack() as ctx:
            sb = ctx.enter_context(tc.tile_pool(name="sb", bufs=1))
            sbs = ctx.enter_context(tc.tile_pool(name="sbs", bufs=2))
            ps = ctx.enter_context(tc.tile_pool(name="ps", bufs=3, space="PSUM"))
            psd = ctx.enter_context(tc.tile_pool(name="psd", bufs=4, space="PSUM"))

            # persistent state
            t_agt = sb.tile([128, G * 128 * NA], F32)
            t_agb = sb.tile([128, G * NA * D], F32)
            t_ag2t = sb.tile([128, G * 128 * NA], F32)
            t_gg = sb.tile([128, G * NA * NT], F32)
            t_scb = sb.tile([128, G * NA * NT], F32)
            t_outs = sb.tile([128, G * NA * NT], F32)
            t_nonag = sb.tile([128, G * NT], F32)
            t_a01 = sb.tile([128, 2 * G * NA], F32)
            t_counts = sb.tile([128, G * NT], F32)
            t_wct = sb.tile([128, 2], F32)
            t_w1 = sb.tile([128, 128], F32)
            t_w2 = sb.tile([128, 128], F32)
            t_bupd = sb.tile([128, 1], F32)
            t_iotak = sb.tile([128, NT], F32)
            t_bc16 = sb.tile([128, G], F32)
            t_ident = sb.tile([128, 128], F32)
            t_ulz = sb.tile([128, G * NA], F32)

            def ap_of(t, extra_off, dims):
                a = t[:]
                return AP(a.tensor, a.offset + extra_off, dims)

            # ---------- prologue ----------
            nc.sync.dma_start(t_agt[:], d_agt.ap())
            nc.sync.dma_start(t_scb[:], d_dot0.ap())
            nc.sync.dma_start(t_a01[:], d_a01.ap())
            nc.sync.dma_start(t_agb[:], d_agb.ap())
            nc.sync.dma_start(t_gg[:], d_gg.ap())
            nc.sync.dma_start(t_nonag[:], d_nonag.ap())
            nc.sync.dma_start(t_wct[:], d_wct.ap())
            nc.sync.dma_start(t_w1[:], d_w1.ap())
            nc.sync.dma_start(t_w2[:], d_w2.ap())
            nc.sync.dma_start(t_bupd[:], d_bupd.ap())
            nc.sync.dma_start(t_iotak[:], d_iotak.ap())
            nc.sync.dma_start(t_bc16[:], d_bc16.ap())
            nc.sync.dma_start(t_ident[:], d_ident.ap())
            nc.sync.dma_start(d_tework.ap(), d_terows.ap())
            nc.vector.memset(t_counts[:], 0.0)
            # scale dot0 and a01 by 1/sqrt(D)
            nc.vector.tensor_scalar(t_scb[:], t_scb[:], INV_SCALE, None,
                                    alu.mult)
            nc.vector.tensor_scalar(t_a01[:], t_a01[:], INV_SCALE, None,
                                    alu.mult)
            scb_all = ap_of(t_scb, 0, [[G * NA * NT, 128], [NA * NT, G],
                                       [NT, NA], [1, NT]])
            gg_all = ap_of(t_gg, 0, [[G * NA * NT, 128], [NA * NT, G],
                                     [NT, NA], [1, NT]])
            nc.vector.tensor_tensor(scb_all, scb_all, gg_all, alu.add)
            na0 = ap_of(t_nonag, 0, [[G * NT, 128], [NT, G], [0, NA], [1, NT]])
            a0_all = ap_of(t_a01, 0, [[2 * G * NA, 128], [NA, G], [1, NA],
                                      [0, NT]])
            prg = sbs.tile([128, G * NA * NT], F32, tag="tlz")
            prg_ap = ap_of(prg, 0, [[G * NA * NT, 128], [NA * NT, G],
                                    [NT, NA], [1, NT]])
            nc.vector.tensor_tensor(prg_ap, na0, a0_all, alu.mult)
            nc.vector.tensor_tensor(scb_all, scb_all, prg_ap, alu.add)

            # P2: AG2T = W1upd-half2 applied to relu(ag^T), + b_upd
            for ch in range(16):
                agrel = sbs.tile([128, 512], F32, tag="agrel")
                nc.scalar.activation(agrel[:],
                                     t_agt[:][:, ch * 512:(ch + 1) * 512],
                                     act.Relu)
                p2 = ps.tile([128, 512], F32, tag="mm")
                nc.tensor.matmul(p2[:], t_w2[:], agrel[:],
                                 start=True, stop=True)
                nc.scalar.activation(t_ag2t[:][:, ch * 512:(ch + 1) * 512],
                                     p2[:], act.Identity, bias=t_bupd[:])

            # ---------- step loop ----------
            nw = BS // 16  # 32 wrapped idx slots
            for s in range(n_steps):
                sc = sbs.tile([128, G, NT], F32, tag="sc")
                tmp = sbs.tile([128, G, NT], F32, tag="tmp")
                a0s = ap_of(t_a01, s, [[2 * G * NA, 128], [NA, G], [0, NT]])
                a1s = ap_of(t_a01, G * NA + s,
                            [[2 * G * NA, 128], [NA, G], [0, NT]])
                scb_s = ap_of(t_scb, s * NT,
                              [[G * NA * NT, 128], [NA * NT, G], [1, NT]])
                gg_s = ap_of(t_gg, s * NT,
                             [[G * NA * NT, 128], [NA * NT, G], [1, NT]])
                nc.vector.tensor_tensor(tmp[:], t_counts[:].rearrange(
                    "p (g k) -> p g k", k=NT), a1s, alu.mult)
                nc.vector.tensor_tensor(sc[:], tmp[:], scb_s, alu.add)

                mx = sbs.tile([128, G], F32, tag="mx")
                nc.vector.tensor_reduce(mx[:], sc[:], mybir.AxisListType.X,
                                        alu.max)
                oh = ap_of(t_outs, s * NT,
                           [[G * NA * NT, 128], [NA * NT, G], [1, NT]])
                mxb = AP(mx[:].tensor, mx[:].offset, [[G, 128], [1, G], [0, NT]])
                nc.vector.tensor_tensor(oh, sc[:], mxb, alu.is_equal)

                # counts += oh * 0.1  (fused)
                nc.vector.scalar_tensor_tensor(
                    t_counts[:].rearrange("p (g k) -> p g k", k=NT), oh, CNF,
                    t_counts[:].rearrange("p (g k) -> p g k", k=NT),
                    alu.mult, alu.add)

                # row idx = b*16 + k*
                iob = AP(t_iotak[:].tensor, t_iotak[:].offset,
                         [[NT, 128], [0, G], [1, NT]])
                nc.vector.tensor_tensor(tmp[:], oh, iob, alu.mult)
                kidx = sbs.tile([128, G], F32, tag="kidx")
                nc.vector.tensor_reduce(kidx[:], tmp[:], mybir.AxisListType.X,
                                        alu.add)
                idxf = sbs.tile([128, G], F32, tag="idxf")
                nc.vector.tensor_tensor(idxf[:], kidx[:], t_bc16[:], alu.add)
                idx16 = sbs.tile([128, G], I16, tag="idx16")
                nc.vector.tensor_copy(idx16[:], idxf[:])

                # wrap to [16, 32] at (q, g*8+ph), then replicate to 128 rows
                idxw = sbs.tile([128, nw], I16, tag="idxw")
                for ph in range(8):
                    src_w = AP(idx16[:].tensor, idx16[:].offset + ph * 16 * G,
                               [[G, 16], [1, G]])        # (q, g)
                    dst_w = AP(idxw[:].tensor, idxw[:].offset + ph,
                               [[nw, 16], [8, G]])       # (q, g)
                    nc.sync.dma_start(dst_w, src_w)
                for npart in (16, 32, 64):
                    src_r = AP(idxw[:].tensor, idxw[:].offset,
                               [[nw, npart], [1, nw]])
                    dst_r = AP(idxw[:].tensor, idxw[:].offset + npart * nw,
                               [[nw, npart], [1, nw]])
                    nc.sync.dma_start(dst_r, src_r)

                # gather selected rows
                r_b = sbs.tile([128, G, D], F32, tag="r_b")
                nc.gpsimd.dma_gather(r_b[:], d_tework.ap(), idxw[:],
                                     num_idxs=BS, num_idxs_reg=BS,
                                     elem_size=D, queue_num=0)

                # relu (b-layout), transpose, upd matmul
                rl_b = sbs.tile([128, G, D], F32, tag="rl_b")
                nc.scalar.activation(rl_b[:], r_b[:], act.Relu)
                rlt = sbs.tile([128, G * 128], F32, tag="rlt")
                for g in range(G):
                    ptr = ps.tile([128, 512], F32, tag="mm")
                    nc.tensor.transpose(ptr[:][:, 0:128], rl_b[:][:, g, :],
                                        t_ident[:])
                    nc.scalar.activation(rlt[:][:, g * 128:(g + 1) * 128],
                                         ptr[:][:, 0:128], act.Identity)
                pu = ps.tile([128, 512], F32, tag="mm")
                nc.tensor.matmul(pu[:], t_w1[:], rlt[:], start=True, stop=True)
                updt = sbs.tile([128, G * 128], F32, tag="updt")
                ag2_s = ap_of(t_ag2t, s, [[G * 128 * NA, 128], [NA, G * 128]])
                nc.vector.tensor_tensor(updt[:], pu[:], ag2_s, alu.add)

                # upd -> b layout, scatter-add into DRAM te rows
                upd_b = sbs.tile([128, G, D], F32, tag="upd_b")
                for g in range(G):
                    ptu = ps.tile([128, 512], F32, tag="mm")
                    nc.tensor.transpose(ptu[:][:, 0:128],
                                        updt[:][:, g * 128:(g + 1) * 128],
                                        t_ident[:])
                    nc.scalar.activation(upd_b[:][:, g, :], ptu[:][:, 0:128],
                                         act.Identity)
                nc.gpsimd.dma_scatter_add(d_tework.ap(), upd_b[:], idxw[:],
                                          num_idxs=BS, num_idxs_reg=BS,
                                          elem_size=D, queue_num=0)

                if s == n_steps - 1:
                    break

                if skip_corr:
                    continue
                # urgent column t'=s+1 first, lazy cols after: lets the
                # scheduler hoist step s+1's score/DMA chain over lazy work
                lzp = sbs.tile([128, NA * D], F32, tag="lzp")
                for (lo, hi) in ((s + 1, s + 2), (s + 2, NA)):
                    ncol = hi - lo
                    if ncol <= 0:
                        continue
                    for g in range(G):
                        in0 = ap_of(upd_b, g * D,
                                    [[G * D, 128], [0, ncol], [1, D]])
                        in1 = ap_of(t_agb, g * NA * D + lo * D,
                                    [[G * NA * D, 128], [D, ncol], [1, D]])
                        lz3 = ap_of(lzp, 0, [[NA * D, 128], [D, ncol], [1, D]])
                        nc.vector.scalar_tensor_tensor(
                            lz3, in0, INV_SCALE, in1, alu.mult, alu.mult)
                        nc.vector.tensor_reduce(
                            t_ulz[:][:, g * NA:g * NA + ncol], lz3,
                            mybir.AxisListType.X, alu.add)
                    scb_u = ap_of(t_scb, lo * NT,
                                  [[G * NA * NT, 128], [NA * NT, G],
                                   [NT, ncol], [1, NT]])
                    ohb = ap_of(t_outs, s * NT,
                                [[G * NA * NT, 128], [NA * NT, G],
                                 [0, ncol], [1, NT]])
                    ulzb = ap_of(t_ulz, 0,
                                 [[G * NA, 128], [NA, G], [1, ncol], [0, NT]])
                    tlz = sbs.tile([128, G * NA * NT], F32, tag="tlz")
                    tlz_ap = ap_of(tlz, 0, [[G * NA * NT, 128], [NA * NT, G],
                                            [NT, ncol], [1, NT]])
                    nc.vector.tensor_tensor(tlz_ap, ohb, ulzb, alu.mult)
                    nc.vector.tensor_tensor(scb_u, scb_u, tlz_ap, alu.add)

            nc.sync.dma_start(d_out.ap(), t_outs[:])

    nc.compile()
    return nc


def _get_nc():
    if "nc" not in _CACHE:
        _CACHE["nc"] = _build()
    return _CACHE["nc"]


def host_inputs(task_embeds, task_nonag_counts, agent_embeds, gumbels,
                W_count, W_upd, b_upd):
    iotak = np.broadcast_to(np.arange(NT, dtype=np.float32), (128, NT)).copy()
    ident = np.eye(128, dtype=np.float32)
    bc16 = ((np.arange(G)[None, :] * 128 + np.arange(128)[:, None]) * NT
            ).astype(np.float32)
    w1 = np.ascontiguousarray(W_upd[:D])
    w2 = np.ascontiguousarray(W_upd[D:])
    wct = np.ascontiguousarray(W_count.T)
    bupd = np.ascontiguousarray(b_upd[:, None])
    maps = []
    for c in range(CORES):
        sl = slice(c * BS, (c + 1) * BS)
        te = task_embeds[sl]
        ag = agent_embeds[sl]
        gum = gumbels[:, sl, :]
        te_g = te.reshape(G, 128, NT, D)
        ag_g = ag.reshape(G, 128, NA, D)
        maps.append(dict(
            terows=np.ascontiguousarray(te.reshape(BS * NT, D)),
            dot0=np.ascontiguousarray(
                np.einsum('btd,bkd->btk', ag, te).reshape(G, 128, NA, NT)
                .transpose(1, 0, 2, 3).reshape(128, G * NA * NT)),
            a01=np.ascontiguousarray(
                np.einsum('btd,jd->bjt', ag, W_count).reshape(G, 128, 2, NA)
                .transpose(1, 2, 0, 3).reshape(128, 2 * G * NA)),
            agt=np.ascontiguousarray(
                ag_g.transpose(3, 0, 1, 2).reshape(128, G * 128 * NA)),
            agb=np.ascontiguousarray(
                ag_g.transpose(1, 0, 2, 3).reshape(128, G * NA * D)),
            gg=np.ascontiguousarray(
                gum.reshape(NA, G, 128, NT).transpose(2, 1, 0, 3)
                .reshape(128, G * NA * NT)),
            nonag=np.ascontiguousarray(
                task_nonag_counts[sl].reshape(G, 128, NT).transpose(1, 0, 2)
                .reshape(128, G * NT)),
            wct=wct, w1=w1, w2=w2, bupd=bupd,
            iotak=iotak, bc16=bc16, ident=ident,
        ))
    return maps


def unshard_out(results):
    out = np.empty((B, NA, NT), dtype=np.float32)
    for c in range(CORES):
        o = results[c]["out"].reshape(128, G, NA, NT)
        out[c * BS:(c + 1) * BS] = o.transpose(1, 0, 2, 3).reshape(BS, NA, NT)
    return out


def kernel(task_embeds, task_nonag_counts, agent_embeds, task_mask,
           agent_mask, gumbels, W_count, b_count, W_upd, b_upd):
    task_embeds = np.asarray(task_embeds, dtype=np.float32)
    task_nonag_counts = np.asarray(task_nonag_counts, dtype=np.float32)
    agent_embeds = np.asarray(agent_embeds, dtype=np.float32)
    gumbels = np.asarray(gumbels, dtype=np.float32)
    W_count = np.asarray(W_count, dtype=np.float32)
    W_upd = np.asarray(W_upd, dtype=np.float32)
    b_upd = np.asarray(b_upd, dtype=np.float32)
    nc = _get_nc()
    in_maps = host_inputs(task_embeds, task_nonag_counts, agent_embeds,
                          gumbels, W_count, W_upd, b_upd)
    res = bass_utils.run_bass_kernel_spmd(nc, in_maps,
                                          core_ids=list(range(CORES)))
    return unshard_out(res.results)


if __name__ == "__main__":
    _build()
    print("build ok")



# revision 12
# speedup vs baseline: 1.7549x; 1.7549x over previous
"""Trainium2 Bass kernel for nn_AutoregressiveAllocPolicy (B=4096, NA=NT=16, D=128).

Math per batch elem b, agent step s:
  logits_k = dot(ag_s, te_k + nonag_k*W0 + counts_k*W1 + b_cnt) / sqrt(D)
  k* = argmax(logits + gumbel_s); out[s] = one_hot(k*)
  counts[k*] += 0.1;  te[k*] += relu([te[k*]; ag_s]) @ W_upd + b_upd

Exploited structure:
  - forward output is exactly one_hot(argmax)  (hard - sg(soft) + soft)
  - b_cnt shifts every k equally -> drop (argmax invariant)
  - te update touches one row/step -> te rows live in DRAM; selected rows
    move via dma_gather / dma_scatter_add (data-dependent row indices)
  - score state SCB[b,t,k] = dot(ag_t, te_cur[b,k])/sqrt(D) kept incrementally:
    initialized ON DEVICE via 512 small PE matmuls (batched per-sample
    ag @ te^T), then per-step corrections add dot(ag_t', upd) deltas.
  - host->device I/O minimized: only te rows, one ag layout, gumbels,
    nonag + tiny weights ship; all other layouts (ag transpose, dot0,
    a01 = ag@W_count^T) are derived on device. Output is the argmax
    index per (b, step) (tiny), expanded to one-hot on the host.

Layout per core: 512 batch elems, b_local = g*128 + p (p partition, g=0..3).
"""
import sys
sys.path.insert(0, '/opt/trn_rl_repo')
import contextlib
import numpy as np

from concourse import bass, mybir, bacc, tile, bass_utils
from concourse.ap import AP

B, NA, NT, D = 4096, 16, 16, 128
CORES = 8
BS = B // CORES          # 512
G = BS // 128            # 4
NC18 = NT + 2            # dot0 matmul rhs cols: 16 te rows + 2 W_count cols
INV_SCALE = float(1.0 / np.sqrt(np.float32(D)))
CNF = 0.1
F32 = mybir.dt.float32
I16 = mybir.dt.int16

_CACHE = {}


def _build(n_steps=NA, skip_corr=False, skip_lazy=False):
    alu = mybir.AluOpType
    act = mybir.ActivationFunctionType
    nc = bacc.Bacc("TRN2", target_bir_lowering=False, debug=False,
                   num_devices=CORES)

    d_terows = nc.dram_tensor("terows", [BS * NT, D], F32, kind="ExternalInput")
    d_agb = nc.dram_tensor("agb", [128, G * NA * D], F32, kind="ExternalInput")
    d_gg = nc.dram_tensor("gg", [128, G * NA * NT], F32, kind="ExternalInput")
    d_nonag = nc.dram_tensor("nonag", [128, G * NT], F32, kind="ExternalInput")
    d_wct = nc.dram_tensor("wct", [128, 2], F32, kind="ExternalInput")
    d_w1 = nc.dram_tensor("w1", [128, 128], F32, kind="ExternalInput")
    d_w2 = nc.dram_tensor("w2", [128, 128], F32, kind="ExternalInput")
    d_bupd = nc.dram_tensor("bupd", [128, 1], F32, kind="ExternalInput")
    d_iotak = nc.dram_tensor("iotak", [128, NT], F32, kind="ExternalInput")
    d_bc16 = nc.dram_tensor("bc16", [128, G], F32, kind="ExternalInput")
    d_ident = nc.dram_tensor("ident", [128, 128], F32, kind="ExternalInput")
    d_oidx = nc.dram_tensor("oidx", [128, NA * G], F32, kind="ExternalOutput")
    d_tework = nc.dram_tensor("tework", [BS * NT, D], F32)
    # DRAM staging for the dot0 shuffle: SBUF-side DMA APs must keep the
    # partition dim outermost, so the (i,t)->(i,j) partition permutation
    # goes through DRAM images of scb / a01 (DRAM APs are unconstrained).
    d_stage = nc.dram_tensor("stage", [128, G * NA * NT], F32)
    d_stga = nc.dram_tensor("stga", [128, 2 * G * NA], F32)

    WTED = G * 128 * NC18    # t_ted free width (9216)
    WAGD = G * 128 * NA      # t_agd free width (8192)

    with tile.TileContext(nc) as tc:
        with contextlib.ExitStack() as ctx:
            sb = ctx.enter_context(tc.tile_pool(name="sb", bufs=1))
            sbs = ctx.enter_context(tc.tile_pool(name="sbs", bufs=2))
            sb1 = ctx.enter_context(tc.tile_pool(name="sb1", bufs=1))
            ps = ctx.enter_context(tc.tile_pool(name="ps", bufs=3, space="PSUM"))
            psd = ctx.enter_context(tc.tile_pool(name="psd", bufs=4, space="PSUM"))

            # persistent state
            t_agb = sb.tile([128, G * NA * D], F32)
            t_agd = sb.tile([128, WAGD], F32)   # [d, (g, p, t)]
            t_ted = sb.tile([128, WTED], F32)   # [d, (g, p, k|wct)]
            t_ag2t = sb.tile([128, G * 128 * NA], F32)
            t_gg = sb.tile([128, G * NA * NT], F32)
            t_scb = sb.tile([128, G * NA * NT], F32)
            t_nonag = sb.tile([128, G * NT], F32)
            t_a01 = sb.tile([128, 2 * G * NA], F32)  # [p, (j, g, t)]
            t_counts = sb.tile([128, G * NT], F32)
            t_wct = sb.tile([128, 2], F32)
            t_w1 = sb.tile([128, 128], F32)
            t_w2 = sb.tile([128, 128], F32)
            t_bupd = sb.tile([128, 1], F32)
            t_iotak = sb.tile([128, NT], F32)
            t_bc16 = sb.tile([128, G], F32)
            t_ident = sb.tile([128, 128], F32)
            t_ulz = sb.tile([128, G * NA], F32)
            t_oidx = sb.tile([128, NA * G], F32)  # [p, (s, g)]

            def ap_of(t, extra_off, dims):
                a = t[:]
                return AP(a.tensor, a.offset + extra_off, dims)

            # ---------- prologue ----------
            nc.sync.dma_start(t_agb[:], d_agb.ap())
            nc.sync.dma_start(t_gg[:], d_gg.ap())
            nc.sync.dma_start(t_nonag[:], d_nonag.ap())
            nc.sync.dma_start(t_wct[:], d_wct.ap())
            nc.sync.dma_start(t_w1[:], d_w1.ap())
            nc.sync.dma_start(t_w2[:], d_w2.ap())
            nc.sync.dma_start(t_bupd[:], d_bupd.ap())
            nc.sync.dma_start(t_iotak[:], d_iotak.ap())
            nc.sync.dma_start(t_bc16[:], d_bc16.ap())
            nc.sync.dma_start(t_ident[:], d_ident.ap())
            nc.sync.dma_start(d_tework.ap(), d_terows.ap())
            nc.vector.memset(t_counts[:], 0.0)

            # Sample p = i*16 + j is assigned to matmul block j (i in 0..8),
            # so the shuffle DMAs below step dst partitions by exactly 1.
            # t_ted col(g, p, n) = g*2304 + j*144 + i*18 + n
            # t_agd col(g, p, t) = g*2048 + j*128 + i*16 + t
            # te rows -> t_ted via 64 PE transposes.  terows row
            # r = g*2048 + p*16 + k; transpose block jj of 128 rows has
            # g, p in [8*jj, +8), all k; for those p: j = 8*(jj%2)+ps,
            # i = jj//2.
            tero = d_terows.ap()
            for jb in range(64):
                g, jj = jb // 16, jb % 16
                st = sbs.tile([128, 128], F32, tag="st")
                nc.sync.dma_start(
                    st[:], AP(tero.tensor, tero.offset + jb * 128 * 128,
                              [[128, 128], [1, 128]]))
                ptr = ps.tile([128, 512], F32, tag="mm")
                nc.tensor.transpose(ptr[:][:, 0:128], st[:], t_ident[:])
                src = ap_of(ptr, 0, [[512, 128], [16, 8], [1, 16]])
                dst = ap_of(t_ted, g * 128 * NC18 + (jj % 2) * 8 * 144
                            + (jj // 2) * NC18,
                            [[WTED, 128], [144, 8], [1, 16]])
                nc.scalar.activation(dst, src, act.Identity)
            # W_count columns appended at n = 16|17 for all (g, p)
            for g in range(G):
                nc.vector.tensor_copy(
                    ap_of(t_ted, g * 128 * NC18 + NT,
                          [[WTED, 128], [144, 16], [NC18, 8], [1, 2]]),
                    ap_of(t_wct, 0, [[2, 128], [0, 16], [0, 8], [1, 2]]))

            # ag -> t_agd via 64 PE transposes (per (g, t), all 128 p).
            for jb in range(64):
                g, t = jb // 16, jb % 16
                ptr = ps.tile([128, 512], F32, tag="mm")
                nc.tensor.transpose(
                    ptr[:][:, 0:128],
                    t_agb[:][:, g * NA * D + t * D:g * NA * D + (t + 1) * D],
                    t_ident[:])
                # psum col p = i*16 + j -> dst col j*128 + i*16 (+t)
                src = ap_of(ptr, 0, [[512, 128], [16, 8], [1, 16]])
                dst = ap_of(t_agd, g * 128 * NA + t,
                            [[WAGD, 128], [16, 8], [128, 16]])
                nc.scalar.activation(dst, src, act.Identity)

            # P2: AG2T = W_upd-half2 applied to relu(ag^T), + b_upd
            for ch in range(16):
                agrel = sbs.tile([128, 512], F32, tag="agrel")
                nc.scalar.activation(agrel[:],
                                     t_agd[:][:, ch * 512:(ch + 1) * 512],
                                     act.Relu)
                p2 = ps.tile([128, 512], F32, tag="mm")
                nc.tensor.matmul(p2[:], t_w2[:], agrel[:],
                                 start=True, stop=True)
                nc.scalar.activation(t_ag2t[:][:, ch * 512:(ch + 1) * 512],
                                     p2[:], act.Identity, bias=t_bupd[:])

            # dot0 + a01 via 64 block matmuls: per (g, j) block of 8
            # samples, out[(i,t), (i',n)] = sum_d ag[d,(i,t)] * ted[d,(i',n)]
            # (n in 0..17 = 16 te rows + 2 W_count cols).  Only the 8
            # diagonal i==i' blocks are used; per-i shuffle DMAs land them
            # as scb [p, (g,t,k)] and a01 [p, (t,g,j)].
            WYG = 16 * 8 * NC18  # 2304
            for g in range(G):
                yg = sb1.tile([128, WYG], F32, tag="yg")
                for j in range(16):
                    pj = psd.tile([128, 512], F32, tag="dot")
                    lhsT = ap_of(t_agd, g * 128 * NA + j * 128,
                                 [[WAGD, 128], [1, 128]])
                    rhs = ap_of(t_ted, g * 128 * NC18 + j * 8 * NC18,
                                [[WTED, 128], [1, 8 * NC18]])
                    nc.tensor.matmul(pj[:][:, 0:8 * NC18], lhsT, rhs,
                                     start=True, stop=True)
                    nc.scalar.activation(
                        yg[:][:, j * 8 * NC18:(j + 1) * 8 * NC18],
                        pj[:][:, 0:8 * NC18], act.Identity)
                stg = d_stage.ap()
                sta = d_stga.ap()
                for i in range(8):
                    # scb image: iterate (t, j, k); src partition i*16+t at
                    # col j*144 + i*18 + k; dst DRAM row i*16+j,
                    # col g*256 + t*16 + k
                    nc.sync.dma_start(
                        AP(stg.tensor,
                           stg.offset + i * 16 * G * NA * NT + g * NA * NT,
                           [[NT, 16], [G * NA * NT, 16], [1, 16]]),
                        ap_of(yg, i * (16 * WYG + NC18),
                              [[WYG, 16], [8 * NC18, 16], [1, 16]]))
                    # a01 image: iterate (t, j, n); dst DRAM row i*16+j,
                    # col t*8 + g*2 + n
                    nc.sync.dma_start(
                        AP(sta.tensor,
                           sta.offset + i * 16 * 2 * G * NA + g * 2,
                           [[2 * G, 16], [2 * G * NA, 16], [1, 2]]),
                        ap_of(yg, i * (16 * WYG + NC18) + NT,
                              [[WYG, 16], [8 * NC18, 16], [1, 2]]))
            nc.sync.dma_start(t_scb[:], d_stage.ap())
            nc.sync.dma_start(t_a01[:], d_stga.ap())

            # scale dot0 and a01 by 1/sqrt(D); fold gumbels + nonag*a0
            nc.vector.tensor_scalar(t_scb[:], t_scb[:], INV_SCALE, None,
                                    alu.mult)
            nc.vector.tensor_scalar(t_a01[:], t_a01[:], INV_SCALE, None,
                                    alu.mult)
            scb_all = ap_of(t_scb, 0, [[G * NA * NT, 128], [NA * NT, G],
                                       [NT, NA], [1, NT]])
            gg_all = ap_of(t_gg, 0, [[G * NA * NT, 128], [NA * NT, G],
                                     [NT, NA], [1, NT]])
            nc.vector.tensor_tensor(scb_all, scb_all, gg_all, alu.add)
            na0 = ap_of(t_nonag, 0, [[G * NT, 128], [NT, G], [0, NA], [1, NT]])
            a0_all = ap_of(t_a01, 0, [[2 * G * NA, 128], [2, G], [2 * G, NA],
                                      [0, NT]])
            prg = sb1.tile([128, G * NA * NT], F32, tag="tlz")
            prg_ap = ap_of(prg, 0, [[G * NA * NT, 128], [NA * NT, G],
                                    [NT, NA], [1, NT]])
            nc.vector.tensor_tensor(prg_ap, na0, a0_all, alu.mult)
            nc.vector.tensor_tensor(scb_all, scb_all, prg_ap, alu.add)

            # ---------- step loop ----------
            nw = BS // 16  # 32 wrapped idx slots
            for s in range(n_steps):
                sc = sbs.tile([128, G, NT], F32, tag="sc")
                tmp = sbs.tile([128, G, NT], F32, tag="tmp")
                a1s = ap_of(t_a01, s * 2 * G + 1,
                            [[2 * G * NA, 128], [2, G], [0, NT]])
                scb_s = ap_of(t_scb, s * NT,
                              [[G * NA * NT, 128], [NA * NT, G], [1, NT]])
                nc.vector.tensor_tensor(tmp[:], t_counts[:].rearrange(
                    "p (g k) -> p g k", k=NT), a1s, alu.mult)
                nc.vector.tensor_tensor(sc[:], tmp[:], scb_s, alu.add)

                mx = sbs.tile([128, G], F32, tag="mx")
                nc.vector.tensor_reduce(mx[:], sc[:], mybir.AxisListType.X,
                                        alu.max)
                oht = sbs.tile([128, G, NT], F32, tag="oh")
                oh = oht[:]
                mxb = AP(mx[:].tensor, mx[:].offset, [[G, 128], [1, G], [0, NT]])
                nc.vector.tensor_tensor(oh, sc[:], mxb, alu.is_equal)

                # counts += oh * 0.1  (fused)
                nc.vector.scalar_tensor_tensor(
                    t_counts[:].rearrange("p (g k) -> p g k", k=NT), oh, CNF,
                    t_counts[:].rearrange("p (g k) -> p g k", k=NT),
                    alu.mult, alu.add)

                # row idx = b*16 + k*
                iob = AP(t_iotak[:].tensor, t_iotak[:].offset,
                         [[NT, 128], [0, G], [1, NT]])
                nc.vector.tensor_tensor(tmp[:], oh, iob, alu.mult)
                kidx = sbs.tile([128, G], F32, tag="kidx")
                nc.vector.tensor_reduce(kidx[:], tmp[:], mybir.AxisListType.X,
                                        alu.add)
                nc.vector.tensor_copy(t_oidx[:][:, s * G:(s + 1) * G],
                                      kidx[:])
                idxf = sbs.tile([128, G], F32, tag="idxf")
                nc.vector.tensor_tensor(idxf[:], kidx[:], t_bc16[:], alu.add)
                idx16 = sbs.tile([128, G], I16, tag="idx16")
                nc.vector.tensor_copy(idx16[:], idxf[:])

                # wrap to [16, 32] at (q, g*8+ph), then replicate to 128 rows
                idxw = sbs.tile([128, nw], I16, tag="idxw")
                for ph in range(8):
                    src_w = AP(idx16[:].tensor, idx16[:].offset + ph * 16 * G,
                               [[G, 16], [1, G]])        # (q, g)
                    dst_w = AP(idxw[:].tensor, idxw[:].offset + ph,
                               [[nw, 16], [8, G]])       # (q, g)
                    nc.sync.dma_start(dst_w, src_w)
                for npart in (16, 32, 64):
                    src_r = AP(idxw[:].tensor, idxw[:].offset,
                               [[nw, npart], [1, nw]])
                    dst_r = AP(idxw[:].tensor, idxw[:].offset + npart * nw,
                               [[nw, npart], [1, nw]])
                    nc.sync.dma_start(dst_r, src_r)

                # gather selected rows
                r_b = sbs.tile([128, G, D], F32, tag="r_b")
                nc.gpsimd.dma_gather(r_b[:], d_tework.ap(), idxw[:],
                                     num_idxs=BS, num_idxs_reg=BS,
                                     elem_size=D, queue_num=0)

                # relu (b-layout), transpose, upd matmul
                rl_b = sbs.tile([128, G, D], F32, tag="rl_b")
                nc.scalar.activation(rl_b[:], r_b[:], act.Relu)
                rlt = sbs.tile([128, G * 128], F32, tag="rlt")
                for g in range(G):
                    ptr = ps.tile([128, 512], F32, tag="mm")
                    nc.tensor.transpose(ptr[:][:, 0:128], rl_b[:][:, g, :],
                                        t_ident[:])
                    nc.scalar.activation(rlt[:][:, g * 128:(g + 1) * 128],
                                         ptr[:][:, 0:128], act.Identity)
                pu = ps.tile([128, 512], F32, tag="mm")
                nc.tensor.matmul(pu[:], t_w1[:], rlt[:], start=True, stop=True)
                updt = sbs.tile([128, G * 128], F32, tag="updt")
                # ag2t col(g, p=i*16+j, t=s) = g*2048 + j*128 + i*16 + s;
                # updt/pu iterate (g, i, j) to match their col = g*128 + p
                ag2_s = ap_of(t_ag2t, s, [[G * 128 * NA, 128], [128 * NA, G],
                                          [NA, 8], [128, 16]])
                nc.vector.tensor_tensor(
                    ap_of(updt, 0, [[512, 128], [128, G], [16, 8], [1, 16]]),
                    ap_of(pu, 0, [[512, 128], [128, G], [16, 8], [1, 16]]),
                    ag2_s, alu.add)

                # upd -> b layout, scatter-add into DRAM te rows
                upd_b = sbs.tile([128, G, D], F32, tag="upd_b")
                for g in range(G):
                    ptu = ps.tile([128, 512], F32, tag="mm")
                    nc.tensor.transpose(ptu[:][:, 0:128],
                                        updt[:][:, g * 128:(g + 1) * 128],
                                        t_ident[:])
                    nc.scalar.activation(upd_b[:][:, g, :], ptu[:][:, 0:128],
                                         act.Identity)
                nc.gpsimd.dma_scatter_add(d_tework.ap(), upd_b[:], idxw[:],
                                          num_idxs=BS, num_idxs_reg=BS,
                                          elem_size=D, queue_num=0)

                if s == n_steps - 1:
                    break

                if skip_corr:
                    continue
                # urgent column t'=s+1 first, lazy cols after: lets the
                # scheduler hoist step s+1's score/DMA chain over lazy work
                lzp = sb1.tile([128, NA * D], F32, tag="lzp")
                for (lo, hi) in ((s + 1, s + 2), (s + 2, NA)):
                    ncol = hi - lo
                    if ncol <= 0:
                        continue
                    for g in range(G):
                        in0 = ap_of(upd_b, g * D,
                                    [[G * D, 128], [0, ncol], [1, D]])
                        in1 = ap_of(t_agb, g * NA * D + lo * D,
                                    [[G * NA * D, 128], [D, ncol], [1, D]])
                        lz3 = ap_of(lzp, 0, [[NA * D, 128], [D, ncol], [1, D]])
                        nc.vector.scalar_tensor_tensor(
                            lz3, in0, INV_SCALE, in1, alu.mult, alu.mult)
                        nc.vector.tensor_reduce(
                            t_ulz[:][:, g * NA:g * NA + ncol], lz3,
                            mybir.AxisListType.X, alu.add)
                    scb_u = ap_of(t_scb, lo * NT,
                                  [[G * NA * NT, 128], [NA * NT, G],
                                   [NT, ncol], [1, NT]])
                    ohb = ap_of(oht, 0,
                                [[G * NT, 128], [NT, G],
                                 [0, ncol], [1, NT]])
                    ulzb = ap_of(t_ulz, 0,
                                 [[G * NA, 128], [NA, G], [1, ncol], [0, NT]])
                    tlz = sb1.tile([128, G * NA * NT], F32, tag="tlz")
                    tlz_ap = ap_of(tlz, 0, [[G * NA * NT, 128], [NA * NT, G],
                                            [NT, ncol], [1, NT]])
                    nc.vector.tensor_tensor(tlz_ap, ohb, ulzb, alu.mult)
                    nc.vector.tensor_tensor(scb_u, scb_u, tlz_ap, alu.add)

            nc.sync.dma_start(d_oidx.ap(), t_oidx[:])

    nc.compile()
    return nc


def _get_nc():
    if "nc" not in _CACHE:
        _CACHE["nc"] = _build()
    return _CACHE["nc"]


def host_inputs(task_embeds, task_nonag_counts, agent_embeds, gumbels,
                W_count, W_upd, b_upd):
    iotak = np.broadcast_to(np.arange(NT, dtype=np.float32), (128, NT)).copy()
    ident = np.eye(128, dtype=np.float32)
    bc16 = ((np.arange(G)[None, :] * 128 + np.arange(128)[:, None]) * NT
            ).astype(np.float32)
    w1 = np.ascontiguousarray(W_upd[:D])
    w2 = np.ascontiguousarray(W_upd[D:])
    wct = np.ascontiguousarray(W_count.T)
    bupd = np.ascontiguousarray(b_upd[:, None])
    maps = []
    for c in range(CORES):
        sl = slice(c * BS, (c + 1) * BS)
        te = task_embeds[sl]
        ag = agent_embeds[sl]
        gum = gumbels[:, sl, :]
        ag_g = ag.reshape(G, 128, NA, D)
        maps.append(dict(
            terows=np.ascontiguousarray(te.reshape(BS * NT, D)),
            agb=np.ascontiguousarray(
                ag_g.transpose(1, 0, 2, 3).reshape(128, G * NA * D)),
            gg=np.ascontiguousarray(
                gum.reshape(NA, G, 128, NT).transpose(2, 1, 0, 3)
                .reshape(128, G * NA * NT)),
            nonag=np.ascontiguousarray(
                task_nonag_counts[sl].reshape(G, 128, NT).transpose(1, 0, 2)
                .reshape(128, G * NT)),
            wct=wct, w1=w1, w2=w2, bupd=bupd,
            iotak=iotak, bc16=bc16, ident=ident,
        ))
    return maps


def unshard_out(results):
    out = np.zeros((B, NA, NT), dtype=np.float32)
    flat = out.reshape(B * NA, NT)
    for c in range(CORES):
        o = results[c]["oidx"].reshape(128, NA, G)  # [p, s, g]
        idx = np.clip(np.rint(o.transpose(2, 0, 1)).astype(np.int64),
                      0, NT - 1)                     # [g, p, s]
        rows = (c * BS + np.arange(BS)[:, None]) * NA + np.arange(NA)[None, :]
        flat[rows.ravel(), idx.reshape(BS * NA).ravel()] = 1.0
    return out


def kernel(task_embeds, task_nonag_counts, agent_embeds, task_mask,
           agent_mask, gumbels, W_count, b_count, W_upd, b_upd):
    task_embeds = np.asarray(task_embeds, dtype=np.float32)
    task_nonag_counts = np.asarray(task_nonag_counts, dtype=np.float32)
    agent_embeds = np.asarray(agent_embeds, dtype=np.float32)
    gumbels = np.asarray(gumbels, dtype=np.float32)
    W_count = np.asarray(W_count, dtype=np.float32)
    W_upd = np.asarray(W_upd, dtype=np.float32)
    b_upd = np.asarray(b_upd, dtype=np.float32)
    nc = _get_nc()
    in_maps = host_inputs(task_embeds, task_nonag_counts, agent_embeds,
                          gumbels, W_count, W_upd, b_upd)
    res = bass_utils.run_bass_kernel_spmd(nc, in_maps,
                                          core_ids=list(range(CORES)))
    return unshard_out(res.results)


if __name__ == "__main__":
    _build()
    print("build ok")
